# revision 11
# baseline (speedup 1.0000x reference)
"""Trainium2 Bass kernel for nn_BipartPool (bipartite GATv2 pooling).

Math (per graph g, centroid r, head h):
    x_l = x @ W_l + b_l                      (source transform, [N, 4, 64])
    x_r = xcent_base @ W_r + b_r             (same for every graph, [16, 4, 64])
    logit[i,r,h] = sum_c lrelu(x_l[i,h,c] + x_r[r,h,c]) * att[h,c]
    alpha = softmax over nodes i of graph g  (per (g,r,h))
    xcent[g*16+r] = mean_h sum_i alpha * x_l[i,h,:] + bias

Sharding: 32 graphs x 1024 nodes, node-aligned -> 4 graphs / core on 8 cores,
zero cross-core communication.

Per-core device algorithm (feature-transposed layout, fp16 logit path):
    XLT[hc, i] = (x_aug @ W_aug)^T           (fp16 PE matmuls; x_aug has ones col)
    relu_r = max(XLT + xrb_r, 0)             (DVE tensor_scalar add+max, 4x mode)
    lrelu decomposition: lrelu(z) = 0.2*z + 0.8*relu(z), so
    logitT[(r,h), i] = A_stack^T @ [XLT; relu_0; ...; relu_15]  (PE, fp16, f32 accum)
      (the per-(r,h) constant 0.2*att.xrb_r is dropped -- softmax shift-invariant)
    softmax along free dim (DVE reduce_max, ACT exp with bias=-m and accum_out=den)
    out[(r,h), c] = ex @ XL / den            (PE fp16; ex transposed via PE)
    xcent = mean_h diag-blocks + bias        (PE transposes + DVE adds)
"""

import numpy as np

N_NODES = 32768
B_GRAPHS = 32
RATIO = 16
C_IN = 64
HEADS = 4
NEG_SLOPE = 0.2
N_CORES = 8
NS = N_NODES // N_CORES          # 4096 nodes per shard
G_PER = B_GRAPHS // N_CORES      # 4 graphs per shard
GN = N_NODES // B_GRAPHS         # 1024 nodes per graph
RH = RATIO * HEADS               # 64 (r,h) pairs
HC = HEADS * C_IN                # 256 features
NKT = 2 + 2 * RATIO              # 34 K-tiles for the logit matmul

_cache = {}


def _build_nc(split_waits=True):
    from contextlib import ExitStack

    import concourse.bass as bass
    import concourse.mybir as mybir
    import concourse.tile as tile

    f16 = mybir.dt.float16
    f32 = mybir.dt.float32
    alu = mybir.AluOpType
    AF = mybir.ActivationFunctionType

    nc = bass.Bass(trn_type="TRN2", target_bir_lowering=False)

    xat_d = nc.dram_tensor("xat", [C_IN + 1, NS], f16, kind="ExternalInput")
    wg_d = nc.dram_tensor("wg", [C_IN + 1, HC], f16, kind="ExternalInput")
    astk_d = nc.dram_tensor("astk", [128, NKT * RH], f16, kind="ExternalInput")
    b16_d = nc.dram_tensor("b16", [128, 2 * RATIO], f32, kind="ExternalInput")
    idn16_d = nc.dram_tensor("idn16", [RH, RH], f16, kind="ExternalInput")
    idn32_d = nc.dram_tensor("idn32", [RH, RH], f32, kind="ExternalInput")
    outb_d = nc.dram_tensor("outb", [C_IN, 1], f32, kind="ExternalInput")
    out_d = nc.dram_tensor("out", [G_PER * RATIO, C_IN], f32, kind="ExternalOutput")

    with tile.TileContext(nc) as tc, ExitStack() as ctx:
        constp = ctx.enter_context(tc.tile_pool(name="const", bufs=1))
        wg_sb = constp.tile([C_IN + 1, HC], f16)
        nc.sync.dma_start(out=wg_sb[:], in_=wg_d[:])
        asb = constp.tile([128, NKT * RH], f16)
        nc.sync.dma_start(out=asb[:], in_=astk_d[:])
        b16 = constp.tile([128, 2 * RATIO], f32)
        nc.sync.dma_start(out=b16[:], in_=b16_d[:])
        idn16 = constp.tile([RH, RH], f16)
        nc.sync.dma_start(out=idn16[:], in_=idn16_d[:])
        idn32 = constp.tile([RH, RH], f32)
        nc.sync.dma_start(out=idn32[:], in_=idn32_d[:])
        outb = constp.tile([C_IN, 1], f32)
        nc.sync.dma_start(out=outb[:], in_=outb_d[:])

        # XLT: transposed transform [256, 4096] fp16, as two partition tiles.
        xlt = [constp.tile([128, NS], f16, tag=f"xlt{p}", name=f"xlt{p}")
               for p in range(2)]
        # XL: natural-layout transform [4096, 256] fp16, node tile t -> cols.
        xlsb = constp.tile([128, (NS // 128) * HC], f16)

        # ---- generation phase: XLT and XL from x_aug^T ----
        with (
            tc.tile_pool(name="gensb", bufs=1) as gensb,
            tc.tile_pool(name="genps", bufs=2, space="PSUM") as genps,
        ):
            xat = gensb.tile([C_IN + 1, NS], f16)
            nc.sync.dma_start(out=xat[:], in_=xat_d[:])
            for c in range(NS // 512):
                for p in range(2):
                    ps = genps.tile([128, 512], f32, tag="g")
                    nc.tensor.matmul(
                        out=ps[:],
                        lhsT=wg_sb[:, 128 * p:128 * (p + 1)],
                        rhs=xat[:, 512 * c:512 * (c + 1)],
                        start=True, stop=True,
                    )
                    nc.scalar.copy(xlt[p][:, 512 * c:512 * (c + 1)], ps[:])
            for tp in range(NS // 256):  # pairs of 128-node tiles
                ps = genps.tile([128, 512], f32, tag="g")
                for u in range(2):
                    t = 2 * tp + u
                    nc.tensor.matmul(
                        out=ps[:, 256 * u:256 * (u + 1)],
                        lhsT=xat[:, 128 * t:128 * (t + 1)],
                        rhs=wg_sb[:],
                        start=True, stop=True,
                    )
                nc.scalar.copy(xlsb[:, 512 * tp:512 * (tp + 1)], ps[:])

        # ---- main phase: per graph ----
        relup = ctx.enter_context(tc.tile_pool(name="relup", bufs=36))
        ltp = ctx.enter_context(tc.tile_pool(name="ltp", bufs=4))
        exp_p = ctx.enter_context(tc.tile_pool(name="exp", bufs=4))
        exnp = ctx.enter_context(tc.tile_pool(name="exnp", bufs=4))
        scalp = ctx.enter_context(tc.tile_pool(name="scalp", bufs=6))
        medp = ctx.enter_context(tc.tile_pool(name="medp", bufs=2))
        psL = ctx.enter_context(tc.tile_pool(name="psL", bufs=3, space="PSUM"))
        psX = ctx.enter_context(tc.tile_pool(name="psX", bufs=2, space="PSUM"))
        psG = ctx.enter_context(tc.tile_pool(name="psG", bufs=1, space="PSUM"))
        psPF = ctx.enter_context(tc.tile_pool(name="psPF", bufs=1, space="PSUM"))

        for g in range(G_PER):
            goff = GN * g
            # relu tiles: relu(XLT + xrb_r) for the graph's 1024 nodes
            rts = {}
            for r in range(RATIO):
                for p in range(2):
                    rt = relup.tile([128, GN], f16, tag="relu")
                    nc.vector.tensor_scalar(
                        out=rt[:],
                        in0=xlt[p][:, goff:goff + GN],
                        scalar1=b16[:, 2 * r + p:2 * r + p + 1],
                        scalar2=0.0,
                        op0=alu.add,
                        op1=alu.max,
                    )
                    rts[(r, p)] = rt

            nms, lt_sbs = [], []
            for ci in range(2):  # 512-node chunks of the graph
                coff = 512 * ci
                lt = psL.tile([RH, 512], f32, tag="lt")
                for k in range(NKT):
                    if k < 2:  # base term: 0.2 * A^T @ XLT
                        rhs = xlt[k][:, goff + coff:goff + coff + 512]
                    else:
                        r, p = (k - 2) // 2, (k - 2) % 2
                        rhs = rts[(r, p)][:, coff:coff + 512]
                    nc.tensor.matmul(
                        out=lt[:],
                        lhsT=asb[:, RH * k:RH * (k + 1)],
                        rhs=rhs,
                        start=(k == 0), stop=(k == NKT - 1),
                    )
                nm = scalp.tile([RH, 1], f32, tag="nm")
                nc.vector.tensor_reduce(
                    out=nm[:], in_=lt[:], axis=mybir.AxisListType.X,
                    op=alu.max, negate=True,
                )
                lt_sb = ltp.tile([RH, 512], f32, tag="ltsb")
                nc.scalar.copy(lt_sb[:], lt[:])
                nms.append(nm)
                lt_sbs.append(lt_sb)

            nmg = scalp.tile([RH, 1], f32, tag="nmg")
            nc.vector.tensor_tensor(out=nmg[:], in0=nms[0][:], in1=nms[1][:], op=alu.min)

            exs, dens = [], []
            for ci in range(2):
                exb = exp_p.tile([RH, 512], f16, tag="ex")
                den = scalp.tile([RH, 1], f32, tag="den")
                nc.scalar.activation(
                    out=exb[:], in_=lt_sbs[ci][:], func=AF.Exp,
                    bias=nmg[:, 0:1], scale=1.0, accum_out=den[:, 0:1],
                )
                exs.append(exb)
                dens.append(den)
            deng = scalp.tile([RH, 1], f32, tag="deng")
            nc.vector.tensor_tensor(out=deng[:], in0=dens[0][:], in1=dens[1][:], op=alu.add)
            rden = scalp.tile([RH, 1], f32, tag="rden")
            nc.vector.reciprocal(rden[:], deng[:])

            # transpose ex -> [128 nodes, 64 rh] tiles
            exns = []
            for ci in range(2):
                px = psX.tile([128, 4 * RH], f16, tag="px")
                for j in range(4):
                    nc.tensor.transpose(
                        out=px[:, RH * j:RH * (j + 1)],
                        in_=exs[ci][:, 128 * j:128 * (j + 1)],
                        identity=idn16[:],
                    )
                exn = exnp.tile([128, 4 * RH], f16, tag="exn")
                nc.scalar.copy(exn[:], px[:])
                exns.append(exn)

            # aggregation: out[(r,h), c] = sum_i ex * XL
            pg = psG.tile([RH, HC], f32, tag="pg")
            for j in range(GN // 128):
                t = (goff // 128) + j
                nc.tensor.matmul(
                    out=pg[:],
                    lhsT=exns[j // 4][:, RH * (j % 4):RH * (j % 4 + 1)],
                    rhs=xlsb[:, HC * t:HC * (t + 1)],
                    start=(j == 0), stop=(j == GN // 128 - 1),
                )
            og = medp.tile([RH, HC], f32, tag="og")
            nc.vector.tensor_scalar_mul(out=og[:], in0=pg[:], scalar1=rden[:, 0:1])

            # mean over heads: transpose to [hc, rh], pick diagonal h-blocks
            pp = psPF.tile([RH, HC], f32, tag="pp")
            for h in range(HEADS):
                nc.tensor.transpose(
                    out=pp[:, RH * h:RH * (h + 1)],
                    in_=og[:, C_IN * h:C_IN * (h + 1)],
                    identity=idn32[:],
                )
            ogt = medp.tile([C_IN, HEADS * RH], f32, tag="ogt")
            nc.vector.tensor_copy(ogt[:], pp[:])
            # term h lives at cols RH*h + (4k + h), k = 0..15
            t01 = medp.tile([C_IN, RATIO], f32, tag="t01")
            t23 = medp.tile([C_IN, RATIO], f32, tag="t23")
            acc = medp.tile([C_IN, RATIO], f32, tag="acc")
            nc.vector.tensor_tensor(
                out=t01[:], in0=ogt[:, 0:61:4], in1=ogt[:, RH + 1:RH + 62:4], op=alu.add)
            nc.vector.tensor_tensor(
                out=t23[:], in0=ogt[:, 2 * RH + 2:2 * RH + 63:4],
                in1=ogt[:, 3 * RH + 3:3 * RH + 64:4], op=alu.add)
            nc.vector.tensor_tensor(out=acc[:], in0=t01[:], in1=t23[:], op=alu.add)
            xct = medp.tile([C_IN, RATIO], f32, tag="xct")
            nc.vector.tensor_scalar(
                out=xct[:], in0=acc[:], scalar1=1.0 / HEADS,
                scalar2=outb[:, 0:1], op0=alu.mult, op1=alu.add,
            )
            pf = psPF.tile([RATIO, C_IN], f32, tag="pf")
            nc.tensor.transpose(out=pf[:], in_=xct[:], identity=idn32[:])
            outt = medp.tile([RATIO, C_IN], f32, tag="outt")
            nc.vector.tensor_copy(outt[:], pf[:])
            nc.sync.dma_start(out=out_d[RATIO * g:RATIO * (g + 1), :], in_=outt[:])

    if split_waits:
        _split_excess_waits(nc, mybir)
    return nc


def _split_excess_waits(nc, mybir):
    """Hoist semaphore waits beyond each ISA struct's single wait slot onto
    standalone EventSemaphore instructions placed just before the owner.

    The cayman MM/ACT/DVE 64B instruction structs carry one EVENTS field
    (1 wait + 1 update); walrus codegen fails with "Too many sync wait
    commands" when Tile's wait assignment leaves 2+ waits on one of them.
    A same-engine standalone wait is semantically identical (engines are
    in-order; the PE reorder window only pulls Ldweights, which keep their
    own wait)."""
    cap_ops = {
        "Matmult", "Ldweights", "Activation", "TensorScalarPtr",
        "TensorTensor", "TensorReduce", "TensorCopy", "Reciprocal",
        "Memset", "TensorScalar", "Iota", "CopyPredicated", "DMACopy",
        "Drain",
    }
    for bb in nc.main_func.blocks:
        out = []
        changed = False
        for ins in bb.instructions:
            si = ins.sync_info
            op = ins.opcode.as_str() if hasattr(ins.opcode, "as_str") else str(ins.opcode)
            if si is not None and op in cap_ops and len(si.on_wait) > 1:
                waits = list(si.on_wait)
                for k, w in enumerate(waits[:-1]):
                    es = mybir.InstEventSemaphore(
                        name=f"{ins.name}-esw{k}", ins=[], outs=[])
                    es.engine = ins.engine
                    es.sync_info = mybir.SyncInfo(on_wait=[w], on_update=[])
                    out.append(es)
                ins.sync_info = mybir.SyncInfo(
                    on_wait=[waits[-1]], on_update=list(si.on_update))
                changed = True
            out.append(ins)
        if changed:
            bb.instructions = out


def _host_prep(x, xcent_base, W_l, b_l, W_r, b_r, att, bias):
    f16 = np.float16
    x = np.asarray(x, np.float32)
    W_l = np.asarray(W_l, np.float32)
    b_l = np.asarray(b_l, np.float32)
    W_r = np.asarray(W_r, np.float32)
    b_r = np.asarray(b_r, np.float32)
    att = np.asarray(att, np.float32)
    bias = np.asarray(bias, np.float32)
    xcent_base = np.asarray(xcent_base, np.float32)

    wg = np.vstack([W_l, b_l[None, :]]).astype(f16)                     # [65, 256]
    xat_full = np.empty((C_IN + 1, N_NODES), f16)
    xat_full[:C_IN] = x.T.astype(f16)
    xat_full[C_IN] = 1.0

    xrb = (xcent_base @ W_r + b_r).astype(f16).astype(np.float32)       # [16, 256]
    b16 = np.empty((128, 2 * RATIO), np.float32)
    for r in range(RATIO):
        for p in range(2):
            b16[:, 2 * r + p] = xrb[r, 128 * p:128 * (p + 1)]

    ablk = np.zeros((HC, HEADS), np.float32)                            # block-diag att
    for h in range(HEADS):
        ablk[h * C_IN:(h + 1) * C_IN, h] = att[h]
    tiles = []
    for p in range(2):                                                  # base: 0.2*A
        tiles.append(np.tile(0.2 * ablk[128 * p:128 * (p + 1), :], (1, RATIO)))
    for r in range(RATIO):                                              # relu: 0.8*A
        for p in range(2):
            t = np.zeros((128, RH), np.float32)
            t[:, 4 * r:4 * (r + 1)] = 0.8 * ablk[128 * p:128 * (p + 1), :]
            tiles.append(t)
    astk = np.concatenate(tiles, axis=1).astype(f16)                    # [128, 34*64]

    idn = np.eye(RH, dtype=np.float32)
    common = {
        "wg": np.ascontiguousarray(wg),
        "astk": np.ascontiguousarray(astk),
        "b16": np.ascontiguousarray(b16),
        "idn16": np.ascontiguousarray(idn.astype(f16)),
        "idn32": np.ascontiguousarray(idn),
        "outb": np.ascontiguousarray(bias[:, None].astype(np.float32)),
    }
    in_maps = []
    for core in range(N_CORES):
        m = dict(common)
        m["xat"] = np.ascontiguousarray(xat_full[:, NS * core:NS * (core + 1)])
        in_maps.append(m)
    return in_maps


def _kernel_numpy(x, xcent_base, W_l, b_l, W_r, b_r, att, bias, batch):
    """Fallback for non-uniform graph sizes (not expected from the reference)."""
    x = np.asarray(x, np.float32)
    xl = (x @ W_l + b_l).reshape(-1, HEADS, C_IN)
    xr = (np.asarray(xcent_base) @ W_r + b_r).reshape(RATIO, HEADS, C_IN)
    T = B_GRAPHS * RATIO
    xcent = np.zeros((T, C_IN), np.float32)
    for gid in range(B_GRAPHS):
        idx = np.nonzero(np.asarray(batch) == gid)[0]
        for r in range(RATIO):
            z = xl[idx] + xr[r]
            z = np.where(z >= 0, z, NEG_SLOPE * z)
            lg = np.einsum('nhc,hc->nh', z, np.asarray(att, np.float32))
            ex = np.exp(lg - lg.max(0))
            al = ex / ex.sum(0)
            out = np.einsum('nhc,nh->hc', xl[idx], al)
            xcent[gid * RATIO + r] = out.mean(0) + np.asarray(bias, np.float32)
    return xcent


def kernel(**inputs):
    x = np.asarray(inputs["x"])
    batch = np.asarray(inputs["batch"])
    args = (x, inputs["xcent_base"], inputs["W_l"], inputs["b_l"],
            inputs["W_r"], inputs["b_r"], inputs["att"], inputs["bias"])

    batchcent = np.repeat(np.arange(B_GRAPHS, dtype=np.int32), RATIO)
    expected_batch = np.repeat(np.arange(B_GRAPHS), N_NODES // B_GRAPHS)
    if x.shape != (N_NODES, C_IN) or not np.array_equal(batch, expected_batch):
        return _kernel_numpy(*args, batch), batchcent

    from concourse.bass_utils import run_bass_kernel_spmd

    if "nc" not in _cache:
        _cache["nc"] = _build_nc()
    in_maps = _host_prep(*args)
    res = run_bass_kernel_spmd(_cache["nc"], in_maps, core_ids=list(range(N_CORES)))
    _cache["last_results"] = res
    xcent = np.concatenate([r["out"] for r in res.results], axis=0).astype(np.float32)
    return xcent, batchcent


# revision 16
# speedup vs baseline: 1.1740x; 1.1740x over previous
"""Trainium2 Bass kernel for nn_BipartPool (bipartite GATv2 pooling).

Math (per graph g, centroid r, head h):
    x_l = x @ W_l + b_l                      (source transform, [N, 4, 64])
    x_r = xcent_base @ W_r + b_r             (same for every graph, [16, 4, 64])
    logit[i,r,h] = sum_c lrelu(x_l[i,h,c] + x_r[r,h,c]) * att[h,c]
    alpha = softmax over nodes i of graph g  (per (g,r,h))
    xcent[g*16+r] = mean_h sum_i alpha * x_l[i,h,:] + bias

Sharding: 32 graphs x 1024 nodes, node-aligned -> 4 graphs / core on 8 cores,
zero cross-core communication.

Per-core device algorithm (feature-transposed layout, fp16 logit path):
    XLT[hc, i] = (x_aug @ W_aug)^T           (fp16 PE matmuls; x_aug has ones col)
    relu_r = max(XLT + xrb_r, 0)             (DVE tensor_scalar add+max, 4x mode)
    lrelu decomposition: lrelu(z) = 0.2*z + 0.8*relu(z), so
    logitT[(r,h), i] = A_stack^T @ [XLT; relu_0; ...; relu_15]  (PE, fp16, f32 accum)
      (the per-(r,h) constant 0.2*att.xrb_r is dropped -- softmax shift-invariant)
    softmax along free dim (DVE reduce_max, ACT exp with bias=-m and accum_out=den)
    out[(r,h), c] = ex @ XL / den            (PE fp16; ex transposed via PE)
    xcent = mean_h diag-blocks + bias        (PE transposes + DVE adds)
"""

import numpy as np

N_NODES = 32768
B_GRAPHS = 32
RATIO = 16
C_IN = 64
HEADS = 4
NEG_SLOPE = 0.2
N_CORES = 8
NS = N_NODES // N_CORES          # 4096 nodes per shard
G_PER = B_GRAPHS // N_CORES      # 4 graphs per shard
GN = N_NODES // B_GRAPHS         # 1024 nodes per graph
RH = RATIO * HEADS               # 64 (r,h) pairs
HC = HEADS * C_IN                # 256 features
NKT = 2 + 2 * RATIO              # 34 K-tiles for the logit matmul

_cache = {}


def _build_nc(split_waits=True):
    from contextlib import ExitStack

    import concourse.bass as bass
    import concourse.mybir as mybir
    import concourse.tile as tile

    f16 = mybir.dt.float16
    f32 = mybir.dt.float32
    alu = mybir.AluOpType
    AF = mybir.ActivationFunctionType

    nc = bass.Bass(trn_type="TRN2", target_bir_lowering=False)

    xat_d = nc.dram_tensor("xat", [C_IN + 1, NS], f16, kind="ExternalInput")
    xnat_d = nc.dram_tensor("xnat", [128, (NS // 128) * (C_IN + 1)], f16,
                            kind="ExternalInput")
    wg_d = nc.dram_tensor("wg", [C_IN + 1, HC], f16, kind="ExternalInput")
    astk_d = nc.dram_tensor("astk", [128, NKT * RH], f16, kind="ExternalInput")
    b16_d = nc.dram_tensor("b16", [128, 2 * RATIO], f32, kind="ExternalInput")
    idn16_d = nc.dram_tensor("idn16", [RH, RH], f16, kind="ExternalInput")
    idn32_d = nc.dram_tensor("idn32", [RH, RH], f32, kind="ExternalInput")
    outb_d = nc.dram_tensor("outb", [C_IN, 1], f32, kind="ExternalInput")
    out_d = nc.dram_tensor("out", [G_PER * RATIO, C_IN], f32, kind="ExternalOutput")

    with tile.TileContext(nc) as tc, ExitStack() as ctx:
        constp = ctx.enter_context(tc.tile_pool(name="const", bufs=1))
        wg_sb = constp.tile([C_IN + 1, HC], f16)
        nc.sync.dma_start(out=wg_sb[:], in_=wg_d[:])

        # XLT: transposed transform [256, 4096] fp16, as two partition tiles.
        xlt = [constp.tile([128, NS], f16, tag=f"xlt{p}", name=f"xlt{p}")
               for p in range(2)]
        # x_aug natural layout: node tile t (128 nodes) -> cols [65t, 65t+65)
        xnat = constp.tile([128, (NS // 128) * (C_IN + 1)], f16)

        # ---- generation phase: XLT from x_aug^T ----
        with (
            tc.tile_pool(name="gensb", bufs=1) as gensb,
            tc.tile_pool(name="genps", bufs=4, space="PSUM") as genps,
        ):
            xat = gensb.tile([C_IN + 1, NS], f16)
            for q in range(4):
                nc.sync.dma_start(out=xat[:, 1024 * q:1024 * (q + 1)],
                                  in_=xat_d[:, 1024 * q:1024 * (q + 1)])
            for c in range(NS // 512):
                for p in range(2):
                    ps = genps.tile([128, 512], f32, tag="g")
                    nc.tensor.matmul(
                        out=ps[:],
                        lhsT=wg_sb[:, 128 * p:128 * (p + 1)],
                        rhs=xat[:, 512 * c:512 * (c + 1)],
                        start=True, stop=True,
                    )
                    eng = nc.scalar if (c + p) % 2 == 0 else nc.vector
                    if eng is nc.scalar:
                        nc.scalar.copy(xlt[p][:, 512 * c:512 * (c + 1)], ps[:])
                    else:
                        nc.vector.tensor_copy(xlt[p][:, 512 * c:512 * (c + 1)], ps[:])

        nc.sync.dma_start(out=xnat[:], in_=xnat_d[:])
        asb = constp.tile([128, NKT * RH], f16)
        nc.sync.dma_start(out=asb[:], in_=astk_d[:])
        b16 = constp.tile([128, 2 * RATIO], f32)
        nc.sync.dma_start(out=b16[:], in_=b16_d[:])
        idn16 = constp.tile([RH, RH], f16)
        nc.sync.dma_start(out=idn16[:], in_=idn16_d[:])
        idn32 = constp.tile([RH, RH], f32)
        nc.sync.dma_start(out=idn32[:], in_=idn32_d[:])
        outb = constp.tile([C_IN, 1], f32)
        nc.sync.dma_start(out=outb[:], in_=outb_d[:])

        # ---- main phase: per graph ----
        relup = ctx.enter_context(tc.tile_pool(name="relup", bufs=40))
        ltp = ctx.enter_context(tc.tile_pool(name="ltp", bufs=4))
        exp_p = ctx.enter_context(tc.tile_pool(name="exp", bufs=4))
        exnp = ctx.enter_context(tc.tile_pool(name="exnp", bufs=4))
        scalp = ctx.enter_context(tc.tile_pool(name="scalp", bufs=6))
        medp = ctx.enter_context(tc.tile_pool(name="medp", bufs=2))
        psL = ctx.enter_context(tc.tile_pool(name="psL", bufs=3, space="PSUM"))
        psX = ctx.enter_context(tc.tile_pool(name="psX", bufs=1, space="PSUM"))
        psG = ctx.enter_context(tc.tile_pool(name="psG", bufs=1, space="PSUM"))
        psPF = ctx.enter_context(tc.tile_pool(name="psPF", bufs=2, space="PSUM"))

        for g in range(G_PER):
            goff = GN * g
            # relu tiles: relu(XLT + xrb_r) for the graph's 1024 nodes
            rts = {}
            for r in range(RATIO):
                for p in range(2):
                    rt = relup.tile([128, GN], f16, tag="relu")
                    nc.vector.tensor_scalar(
                        out=rt[:],
                        in0=xlt[p][:, goff:goff + GN],
                        scalar1=b16[:, 2 * r + p:2 * r + p + 1],
                        scalar2=0.0,
                        op0=alu.add,
                        op1=alu.max,
                    )
                    rts[(r, p)] = rt

            # logitT accumulation, k-outer so each A_stack K-tile's weights
            # load once for both 512-node chunks
            lts = [psL.tile([RH, 512], f32, tag="lt", name=f"lt{g}_{ci}")
                   for ci in range(2)]
            for k in range(NKT):
                for ci in range(2):
                    coff = 512 * ci
                    if k < 2:  # base term: 0.2 * A^T @ XLT
                        rhs = xlt[k][:, goff + coff:goff + coff + 512]
                    else:
                        r, p = (k - 2) // 2, (k - 2) % 2
                        rhs = rts[(r, p)][:, coff:coff + 512]
                    nc.tensor.matmul(
                        out=lts[ci][:],
                        lhsT=asb[:, RH * k:RH * (k + 1)],
                        rhs=rhs,
                        start=(k == 0), stop=(k == NKT - 1),
                    )
            nms, lt_sbs = [], []
            for ci in range(2):
                nm = scalp.tile([RH, 1], f32, tag="nm")
                nc.vector.tensor_reduce(
                    out=nm[:], in_=lts[ci][:], axis=mybir.AxisListType.X,
                    op=alu.max, negate=True,
                )
                lt_sb = ltp.tile([RH, 512], f32, tag="ltsb")
                nc.scalar.copy(lt_sb[:], lts[ci][:])
                nms.append(nm)
                lt_sbs.append(lt_sb)

            nmg = scalp.tile([RH, 1], f32, tag="nmg")
            nc.vector.tensor_tensor(out=nmg[:], in0=nms[0][:], in1=nms[1][:], op=alu.min)

            exs, dens = [], []
            for ci in range(2):
                exb = exp_p.tile([RH, 512], f16, tag="ex")
                den = scalp.tile([RH, 1], f32, tag="den")
                nc.scalar.activation(
                    out=exb[:], in_=lt_sbs[ci][:], func=AF.Exp,
                    bias=nmg[:, 0:1], scale=1.0, accum_out=den[:, 0:1],
                )
                exs.append(exb)
                dens.append(den)
            deng = scalp.tile([RH, 1], f32, tag="deng")
            nc.vector.tensor_tensor(out=deng[:], in0=dens[0][:], in1=dens[1][:], op=alu.add)
            rden = scalp.tile([RH, 1], f32, tag="rden")
            nc.vector.reciprocal(rden[:], deng[:])

            # alpha = ex * rden, then transpose -> [128 nodes, 64 rh] tiles
            exns = []
            for ci in range(2):
                al = exp_p.tile([RH, 512], f16, tag="alpha")
                nc.vector.tensor_scalar_mul(
                    out=al[:], in0=exs[ci][:], scalar1=rden[:, 0:1])
                px = psX.tile([128, 4 * RH], f16, tag="px")
                for j in range(4):
                    nc.tensor.transpose(
                        out=px[:, RH * j:RH * (j + 1)],
                        in_=al[:, 128 * j:128 * (j + 1)],
                        identity=idn16[:],
                    )
                exn = exnp.tile([128, 4 * RH], f16, tag="exn")
                nc.scalar.copy(exn[:], px[:])
                exns.append(exn)

            # aggregation stage 1: exX^T[k, rh] = sum_i x_aug[i, k] alpha[rh, i]
            pg = psG.tile([C_IN + 1, RH], f32, tag="pg")
            for j in range(GN // 128):
                t = (goff // 128) + j
                nc.tensor.matmul(
                    out=pg[:],
                    lhsT=xnat[:, (C_IN + 1) * t:(C_IN + 1) * (t + 1)],
                    rhs=exns[j // 4][:, RH * (j % 4):RH * (j % 4 + 1)],
                    start=(j == 0), stop=(j == GN // 128 - 1),
                )
            ext = medp.tile([C_IN + 1, RH], f16, tag="ext")
            nc.scalar.copy(ext[:], pg[:])
            # stage 2: og[rh, hc] = exX^T.T @ W_aug
            ogp = psPF.tile([RH, HC], f32, tag="pp", name="ogp")
            nc.tensor.matmul(out=ogp[:], lhsT=ext[:], rhs=wg_sb[:],
                             start=True, stop=True)
            og = medp.tile([RH, HC], f32, tag="og")
            nc.vector.tensor_copy(out=og[:], in_=ogp[:])

            # mean over heads: transpose to [hc, rh], pick diagonal h-blocks
            pp = psPF.tile([RH, HC], f32, tag="pp")
            for h in range(HEADS):
                nc.tensor.transpose(
                    out=pp[:, RH * h:RH * (h + 1)],
                    in_=og[:, C_IN * h:C_IN * (h + 1)],
                    identity=idn32[:],
                )
            ogt = medp.tile([C_IN, HEADS * RH], f32, tag="ogt")
            nc.vector.tensor_copy(ogt[:], pp[:])
            # term h lives at cols RH*h + (4k + h), k = 0..15
            t01 = medp.tile([C_IN, RATIO], f32, tag="t01")
            t23 = medp.tile([C_IN, RATIO], f32, tag="t23")
            acc = medp.tile([C_IN, RATIO], f32, tag="acc")
            nc.vector.tensor_tensor(
                out=t01[:], in0=ogt[:, 0:61:4], in1=ogt[:, RH + 1:RH + 62:4], op=alu.add)
            nc.vector.tensor_tensor(
                out=t23[:], in0=ogt[:, 2 * RH + 2:2 * RH + 63:4],
                in1=ogt[:, 3 * RH + 3:3 * RH + 64:4], op=alu.add)
            nc.vector.tensor_tensor(out=acc[:], in0=t01[:], in1=t23[:], op=alu.add)
            xct = medp.tile([C_IN, RATIO], f32, tag="xct")
            nc.vector.tensor_scalar(
                out=xct[:], in0=acc[:], scalar1=1.0 / HEADS,
                scalar2=outb[:, 0:1], op0=alu.mult, op1=alu.add,
            )
            pf = psPF.tile([RATIO, C_IN], f32, tag="pp", name="pf")
            nc.tensor.transpose(out=pf[:], in_=xct[:], identity=idn32[:])
            outt = medp.tile([RATIO, C_IN], f32, tag="outt")
            nc.vector.tensor_copy(outt[:], pf[:])
            nc.sync.dma_start(out=out_d[RATIO * g:RATIO * (g + 1), :], in_=outt[:])

    if split_waits:
        _split_excess_waits(nc, mybir)
    return nc


def _split_excess_waits(nc, mybir):
    """Hoist semaphore waits beyond each ISA struct's single wait slot onto
    standalone EventSemaphore instructions placed just before the owner.

    The cayman MM/ACT/DVE 64B instruction structs carry one EVENTS field
    (1 wait + 1 update); walrus codegen fails with "Too many sync wait
    commands" when Tile's wait assignment leaves 2+ waits on one of them.
    A same-engine standalone wait is semantically identical (engines are
    in-order; the PE reorder window only pulls Ldweights, which keep their
    own wait)."""
    cap_ops = {
        "Matmult", "Ldweights", "Activation", "TensorScalarPtr",
        "TensorTensor", "TensorReduce", "TensorCopy", "Reciprocal",
        "Memset", "TensorScalar", "Iota", "CopyPredicated", "DMACopy",
        "Drain",
    }
    for bb in nc.main_func.blocks:
        out = []
        changed = False
        for ins in bb.instructions:
            si = ins.sync_info
            op = ins.opcode.as_str() if hasattr(ins.opcode, "as_str") else str(ins.opcode)
            if si is not None and op in cap_ops and len(si.on_wait) > 1:
                waits = list(si.on_wait)
                for k, w in enumerate(waits[:-1]):
                    es = mybir.InstEventSemaphore(
                        name=f"{ins.name}-esw{k}", ins=[], outs=[])
                    es.engine = ins.engine
                    es.sync_info = mybir.SyncInfo(on_wait=[w], on_update=[])
                    out.append(es)
                ins.sync_info = mybir.SyncInfo(
                    on_wait=[waits[-1]], on_update=list(si.on_update))
                changed = True
            out.append(ins)
        if changed:
            bb.instructions = out


def _host_prep(x, xcent_base, W_l, b_l, W_r, b_r, att, bias):
    f16 = np.float16
    x = np.asarray(x, np.float32)
    W_l = np.asarray(W_l, np.float32)
    b_l = np.asarray(b_l, np.float32)
    W_r = np.asarray(W_r, np.float32)
    b_r = np.asarray(b_r, np.float32)
    att = np.asarray(att, np.float32)
    bias = np.asarray(bias, np.float32)
    xcent_base = np.asarray(xcent_base, np.float32)

    wg = np.vstack([W_l, b_l[None, :]]).astype(f16)                     # [65, 256]
    xat_full = np.empty((C_IN + 1, N_NODES), f16)
    xat_full[:C_IN] = x.T.astype(f16)
    xat_full[C_IN] = 1.0
    # natural layout, 128-node tiles side by side: [128, 32*65] per core
    xa_nat = np.empty((N_NODES, C_IN + 1), f16)
    xa_nat[:, :C_IN] = x.astype(f16)
    xa_nat[:, C_IN] = 1.0
    xnat_full = np.ascontiguousarray(
        xa_nat.reshape(N_NODES // 128, 128, C_IN + 1).transpose(1, 0, 2)
        .reshape(128, -1))                                              # [128, 256*65]

    xrb = (xcent_base @ W_r + b_r).astype(f16).astype(np.float32)       # [16, 256]
    b16 = np.empty((128, 2 * RATIO), np.float32)
    for r in range(RATIO):
        for p in range(2):
            b16[:, 2 * r + p] = xrb[r, 128 * p:128 * (p + 1)]

    ablk = np.zeros((HC, HEADS), np.float32)                            # block-diag att
    for h in range(HEADS):
        ablk[h * C_IN:(h + 1) * C_IN, h] = att[h]
    tiles = []
    for p in range(2):                                                  # base: 0.2*A
        tiles.append(np.tile(0.2 * ablk[128 * p:128 * (p + 1), :], (1, RATIO)))
    for r in range(RATIO):                                              # relu: 0.8*A
        for p in range(2):
            t = np.zeros((128, RH), np.float32)
            t[:, 4 * r:4 * (r + 1)] = 0.8 * ablk[128 * p:128 * (p + 1), :]
            tiles.append(t)
    astk = np.concatenate(tiles, axis=1).astype(f16)                    # [128, 34*64]

    idn = np.eye(RH, dtype=np.float32)
    common = {
        "wg": np.ascontiguousarray(wg),
        "astk": np.ascontiguousarray(astk),
        "b16": np.ascontiguousarray(b16),
        "idn16": np.ascontiguousarray(idn.astype(f16)),
        "idn32": np.ascontiguousarray(idn),
        "outb": np.ascontiguousarray(bias[:, None].astype(np.float32)),
    }
    npt = NS // 128  # node tiles per core
    in_maps = []
    for core in range(N_CORES):
        m = dict(common)
        m["xat"] = np.ascontiguousarray(xat_full[:, NS * core:NS * (core + 1)])
        m["xnat"] = np.ascontiguousarray(
            xnat_full[:, (C_IN + 1) * npt * core:(C_IN + 1) * npt * (core + 1)])
        in_maps.append(m)
    return in_maps


def _kernel_numpy(x, xcent_base, W_l, b_l, W_r, b_r, att, bias, batch):
    """Fallback for non-uniform graph sizes (not expected from the reference)."""
    x = np.asarray(x, np.float32)
    xl = (x @ W_l + b_l).reshape(-1, HEADS, C_IN)
    xr = (np.asarray(xcent_base) @ W_r + b_r).reshape(RATIO, HEADS, C_IN)
    T = B_GRAPHS * RATIO
    xcent = np.zeros((T, C_IN), np.float32)
    for gid in range(B_GRAPHS):
        idx = np.nonzero(np.asarray(batch) == gid)[0]
        for r in range(RATIO):
            z = xl[idx] + xr[r]
            z = np.where(z >= 0, z, NEG_SLOPE * z)
            lg = np.einsum('nhc,hc->nh', z, np.asarray(att, np.float32))
            ex = np.exp(lg - lg.max(0))
            al = ex / ex.sum(0)
            out = np.einsum('nhc,nh->hc', xl[idx], al)
            xcent[gid * RATIO + r] = out.mean(0) + np.asarray(bias, np.float32)
    return xcent


def kernel(**inputs):
    x = np.asarray(inputs["x"])
    batch = np.asarray(inputs["batch"])
    args = (x, inputs["xcent_base"], inputs["W_l"], inputs["b_l"],
            inputs["W_r"], inputs["b_r"], inputs["att"], inputs["bias"])

    batchcent = np.repeat(np.arange(B_GRAPHS, dtype=np.int32), RATIO)
    expected_batch = np.repeat(np.arange(B_GRAPHS), N_NODES // B_GRAPHS)
    if x.shape != (N_NODES, C_IN) or not np.array_equal(batch, expected_batch):
        return _kernel_numpy(*args, batch), batchcent

    from concourse.bass_utils import run_bass_kernel_spmd

    if "nc" not in _cache:
        _cache["nc"] = _build_nc()
    in_maps = _host_prep(*args)
    res = run_bass_kernel_spmd(_cache["nc"], in_maps, core_ids=list(range(N_CORES)))
    _cache["last_results"] = res
    xcent = np.concatenate([r["out"] for r in res.results], axis=0).astype(np.float32)
    return xcent, batchcent
